# revision 27
# baseline (speedup 1.0000x reference)
"""Channel-attention (CAM) Trainium2 kernel.

Reference computation (per batch b of 16):
    q   = x[b].reshape(C, HW)                  # C=512, HW=4096
    sim = q @ q.T                              # [C, C], symmetric
    attn = softmax(max(sim) - sim, axis=-1)    # == exp(min_r - sim) / Z_r
    out[b] = gamma * attn @ q + x[b]

Sharding: data-parallel over batch across 8 NeuronCores (2 batches/core).
kernel() takes full inputs, shards internally, returns the full output.

Per-core kernel design:
  - All matmuls in float32r (fp32 storage, ~13-bit mantissa, full PE rate
    at N>=256). bf16 is NOT accurate enough here: the softmax is
    winner-take-all (sim entries spread sigma~64), so sim errors ~0.3 flip
    argmin rows. f32r gives end-to-end max_abs_err ~0.018 (rel L2 4e-4).
  - x is streamed in column-waves, rounded to f32r (DVE), transposed on
    the PE (via identity matmuls) into qT tiles [n, c]; sim matmuls run
    two transpose-chunks behind, so DMA/round/transpose/matmul pipeline.
  - sim is symmetric: compute block rows over cols >= (0,128,256,256)
    only, and fill the 5 missing lower [128,128] blocks by PE-transposing
    their mirror blocks out of PSUM.
  - softmax via ACT: p = exp(min_r - sim) with accum_out producing Z in
    the same pass; rows scaled by gamma/Z (DVE), PE-transposed, and the
    identity is added to the diagonal so the second matmul directly
    computes gamma*attn@q + q = out (residual folded into the matmul;
    note x == q here).
  - mm2 results are staged [128, 2048] in SBUF and stored with 8KB
    descriptor lines; input loads use the Sync HWDGE queue, batch-0
    stores the ACT queue, batch-1 stores Sync (idle by then) so store
    sem-waits never block the next batch's loads.
  - 8 dummy identity matmuls at t=0 pre-warm the PE clock gate (HAM)
    while the first loads are in flight.

Measured on trn2 (8 cores, axon): ~169 us HW exec for the full B=16
problem, vs ~125 us warm-PE-roofline for this decomposition.
"""
import sys

if "/opt/trn_rl_repo" not in sys.path:
    sys.path.insert(0, "/opt/trn_rl_repo")

import numpy as np

B, C, H, W = 16, 512, 64, 64
HW = H * W
NCORES = 8
NB = B // NCORES          # batches per core
P = 128
CB = C // P               # 4 channel blocks
KN = HW // P              # 32 contraction chunks for sim
NJ = HW // 512            # 8 output column chunks

_BUILD_CACHE = {}


def build_bass():
    import concourse.bacc as bacc
    import concourse.tile as tile
    from concourse import mybir
    from concourse.masks import make_identity

    f32 = mybir.dt.float32
    f32r = mybir.dt.float32r
    AX = mybir.AxisListType
    ALU = mybir.AluOpType
    ACTF = mybir.ActivationFunctionType

    nc = bacc.Bacc()
    x_ext = nc.declare_dram_parameter("x", [NB, C, HW], f32, isOutput=False)
    g_ext = nc.declare_dram_parameter("gamma", [1], f32, isOutput=False)
    o_ext = nc.declare_dram_parameter("out", [NB, C, HW], f32, isOutput=True)

    # alternate PSUM->SBUF copies between ACT and DVE to balance engines
    _flip = [0]

    with tile.TileContext(nc) as tc:
        with (
            tc.tile_pool(name="const", bufs=1) as const,
            tc.tile_pool(name="xchunk", bufs=5) as xchunk,
            tc.tile_pool(name="qr", bufs=8) as qrp,
            tc.tile_pool(name="qt", bufs=10) as qtp,
            tc.tile_pool(name="pp", bufs=4) as pp,
            tc.tile_pool(name="osb", bufs=2) as osb,
            tc.tile_pool(name="tri", bufs=2) as trip,
            tc.tile_pool(name="vec", bufs=6) as vec,
            tc.tile_pool(name="psA", bufs=2, space="PSUM") as psA,
            tc.tile_pool(name="psim", bufs=4, space="PSUM") as psimp,
            tc.tile_pool(name="pfeat", bufs=2, space="PSUM") as pfeat,
        ):
            def copyback(dst, src):
                if _flip[0] % 2 == 0:
                    nc.scalar.copy(dst, src)
                else:
                    nc.vector.tensor_copy(dst, src)
                _flip[0] += 1

            # batch-0 first-wave loads go first so DMA starts during preamble
            pre_x = {}
            for mi in range(CB):
                xt = xchunk.tile([P, 1024], f32, tag="xc", name=f"prex{mi}")
                nc.sync.dma_start(
                    out=xt[:, :512], in_=x_ext[0, mi * P:(mi + 1) * P, 0:512]
                )
                pre_x[mi] = xt

            ident_f = const.tile([P, P], f32)
            make_identity(nc, ident_f)
            ident_r = const.tile([P, P], f32r)
            nc.vector.tensor_copy(ident_r[:], ident_f[:])
            gamma_sb = const.tile([P, 1], f32)
            nc.sync.dma_start(out=gamma_sb[:], in_=g_ext[:].to_broadcast([P, 1]))
            inv_gamma = const.tile([P, 1], f32)
            nc.vector.reciprocal(inv_gamma[:], gamma_sb[:])

            # dummy matmuls while the first loads land: warms the PE clock
            # gate (HAM) so real matmuls start at full rate
            warm = psA.tile([P, C], f32, tag="psA", name="warmup")
            for i in range(8):
                nc.tensor.matmul(warm[:, :P], ident_f[:], ident_f[:],
                                 start=True, stop=True)

            # column waves per batch; first two finer to cut startup latency
            WAVES = [(0, 512), (512, 512), (1024, 1024), (2048, 1024), (3072, 1024)]

            for b in range(NB):
                qr_t = [qrp.tile([P, HW], f32r, tag="qr", name=f"qr{b}_{i}")
                        for i in range(CB)]
                psim = [psimp.tile([P, C], f32, tag="psim", name=f"psim{b}_{i}")
                        for i in range(CB)]

                c0s = [min(mi * P, 2 * P) for mi in range(CB)]  # 0,128,256,256

                # phase 1+2 pipeline: per wave, load+round columns, transpose
                # to qT, and run sim matmuls one kn behind the transposes.
                qt_tiles = {}

                def mm1(kn):
                    for mi in range(CB):
                        c0 = c0s[mi]
                        nc.tensor.matmul(
                            psim[mi][:, c0:],
                            qt_tiles[kn][:, mi * P:(mi + 1) * P],
                            qt_tiles[kn][:, c0:],
                            start=(kn == 0),
                            stop=(kn == KN - 1),
                        )

                pending = []
                for (w0, wlen) in WAVES:
                    for mi in range(CB):
                        if b == 0 and w0 == 0:
                            xt = pre_x[mi]
                        else:
                            xt = xchunk.tile([P, 1024], f32, tag="xc")
                            nc.sync.dma_start(
                                out=xt[:, :wlen],
                                in_=x_ext[b, mi * P:(mi + 1) * P, w0:w0 + wlen],
                            )
                        nc.vector.tensor_copy(
                            qr_t[mi][:, w0:w0 + wlen], xt[:, :wlen]
                        )
                    for kq in range(wlen // P):
                        kn = w0 // P + kq
                        pst = psA.tile([P, C], f32r, tag="psA")
                        for ci in range(CB):
                            nc.tensor.transpose(
                                pst[:, ci * P:(ci + 1) * P],
                                qr_t[ci][:, kn * P:(kn + 1) * P],
                                ident_r[:],
                            )
                        qt = qtp.tile([P, C], f32r, tag="qt", name=f"qt{b}_{kn}")
                        qt_tiles[kn] = qt
                        copyback(qt[:], pst[:])
                        pending.append(kn)
                        if len(pending) > 2:
                            mm1(pending.pop(0))
                for kn in pending:
                    mm1(kn)

                # ---- fill lower blocks by symmetry: (i, j) = T((j, i)) ----
                for (i, j) in [(1, 0), (2, 0), (2, 1), (3, 0), (3, 1)]:
                    tmp = trip.tile([P, P], f32, tag="tri")
                    nc.scalar.copy(tmp[:], psim[j][:, i * P:(i + 1) * P])
                    nc.tensor.transpose(
                        psim[i][:, j * P:(j + 1) * P], tmp[:], ident_f[:]
                    )

                # ---- softmax rows: p_s = (gamma/Z) * exp(min_r - sim) ----
                ps_t = []
                for mi in range(CB):
                    mrow = vec.tile([P, 1], f32, tag="mrow")
                    nc.vector.tensor_reduce(
                        mrow[:], psim[mi][:], axis=AX.X, op=ALU.min
                    )
                    zrow = vec.tile([P, 1], f32, tag="zrow")
                    p_t = pp.tile([P, C], f32r, tag="p", bufs=2)
                    nc.scalar.activation(
                        p_t[:], psim[mi][:], ACTF.Exp,
                        bias=mrow[:], scale=-1.0, accum_out=zrow[:],
                    )
                    rz = vec.tile([P, 1], f32, tag="rz")
                    nc.vector.reciprocal(rz[:], zrow[:])
                    rzg = vec.tile([P, 1], f32, tag="rzg")
                    nc.vector.tensor_mul(rzg[:], rz[:], gamma_sb[:])
                    p_s = pp.tile([P, C], f32r, tag="psc", bufs=4)
                    nc.vector.tensor_scalar_mul(p_s[:], p_t[:], rzg[:])
                    ps_t.append(p_s)

                # ---- lhsT for mm2: pT = T(p_s) + I ----
                pt_t = []
                for kd in range(CB):
                    pst = pfeat.tile([P, C], f32r, tag="pf")
                    for ci in range(CB):
                        nc.tensor.transpose(
                            pst[:, ci * P:(ci + 1) * P],
                            ps_t[ci][:, kd * P:(kd + 1) * P],
                            ident_r[:],
                        )
                    t = pp.tile([P, C], f32r, tag="pt")
                    copyback(t[:], pst[:])
                    nc.vector.tensor_add(
                        t[:, kd * P:(kd + 1) * P],
                        t[:, kd * P:(kd + 1) * P],
                        ident_r[:],
                    )
                    pt_t.append(t)

                # ---- out = (gamma*diag(1/Z)*P + I) @ q, staged stores ----
                for mi in range(CB):
                    fine = (b == NB - 1 and mi == CB - 1)
                    st_eng = nc.scalar if b == 0 else nc.sync
                    for half in range(2):
                        stg = osb.tile([P, HW // 2], f32, tag="ot")
                        for njh in range(NJ // 2):
                            nj = half * (NJ // 2) + njh
                            pf = pfeat.tile([P, 512], f32, tag="pf")
                            for kd in range(CB):
                                nc.tensor.matmul(
                                    pf[:],
                                    pt_t[kd][:, mi * P:(mi + 1) * P],
                                    qr_t[kd][:, nj * 512:(nj + 1) * 512],
                                    start=(kd == 0),
                                    stop=(kd == CB - 1),
                                )
                            copyback(stg[:, njh * 512:(njh + 1) * 512], pf[:])
                            if fine:
                                st_eng.dma_start(
                                    out=o_ext[b, mi * P:(mi + 1) * P,
                                              nj * 512:(nj + 1) * 512],
                                    in_=stg[:, njh * 512:(njh + 1) * 512],
                                )
                        if not fine:
                            st_eng.dma_start(
                                out=o_ext[b, mi * P:(mi + 1) * P,
                                          half * (HW // 2):(half + 1) * (HW // 2)],
                                in_=stg[:],
                            )

    nc.finalize()
    return nc


def get_bass():
    if "nc" not in _BUILD_CACHE:
        _BUILD_CACHE["nc"] = build_bass()
    return _BUILD_CACHE["nc"]


def make_in_maps(x, gamma):
    x = np.ascontiguousarray(np.asarray(x, dtype=np.float32)).reshape(B, C, HW)
    gamma = np.asarray(gamma, dtype=np.float32).reshape(1)
    return [
        {"x": x[i * NB:(i + 1) * NB], "gamma": gamma}
        for i in range(NCORES)
    ]


def run(x, gamma, trace=False, **trace_kwargs):
    from concourse.bass_utils import run_bass_kernel_spmd

    nc = get_bass()
    res = run_bass_kernel_spmd(
        nc, make_in_maps(x, gamma), core_ids=list(range(NCORES)),
        trace=trace, **trace_kwargs,
    )
    out = np.concatenate([res.results[i]["out"] for i in range(NCORES)], axis=0)
    return out.reshape(B, C, H, W), res


def kernel(x, gamma):
    out, _ = run(x, gamma, trace=False)
    return out
